# revision 23
# baseline (speedup 1.0000x reference)
"""EMA dechunker kernel for Trainium2 (Bass/Tile), 8-core data-parallel.

Problem: for each batch row
  smoothed[j] = m[j] ? clip(p[j])*emb[j] + (1-clip(p[j]))*smoothed[j-1]
                     : smoothed[j-1]
  frames[l]   = smoothed[clip(cumsum(boundary)[l]-1, 0, J-1)]

Sharding: batch dim B=16 split across 8 cores (2 rows/core).

v3 design:
 - EMA as blocked matmuls in NATURAL layout (no PE transposes, no DVE
   scans): within each 128-step chunk, s = L @ x + d * carry where
   L[i,j] = prod_{j<k<=i} a_k = exp(cs[i]-cs[j]) (lower-tri), d[i] =
   exp(cs[i]), cs = cumsum(log a) via a tri128 matmul. L is built with
   one DVE add (row-broadcast + additive -1e9 mask) and one fused ACT
   exp (bias = -cs column, per-partition). The carry enters as a second
   matmul with lhsT nonzero only in partition 127, reading the previous
   chunk's bf16 smoothed tile, closing the PSUM accumulation group.
 - smoothed is bf16 in DRAM (halves gather traffic; tol is 2e-2, bf16
   costs ~0.3%; the f32 PSUM keeps the recurrence accurate).
 - Upsample gather via gpsimd indirect DMA (DGE walks a [128, 8] int32
   row-index tile; 1 KiB descriptors avoid the gather-ucode's slow
   coalesced path), indices in the same (l%128)-partition wrap as the
   output tile.
 - Output stores cast bf16->f32 inside the DMA (gpsimd-initiated DMAs
   can cast), so no compute-engine converts are needed.
"""

from contextlib import ExitStack

import numpy as np

import concourse.bass as bass
import concourse.tile as tile
from concourse import bacc, mybir
from concourse.bass_utils import run_bass_kernel_spmd
from concourse.masks import make_identity

F32 = mybir.dt.float32
BF16 = mybir.dt.bfloat16
I32 = mybir.dt.int32
I16 = mybir.dt.int16
U8 = mybir.dt.uint8
OP = mybir.AluOpType
AF = mybir.ActivationFunctionType

B, J, L, D = 16, 1024, 4096, 512
N_CORES = 8
BL = B // N_CORES          # 2 batch rows per core
T = 128                    # j-chunk size (PSUM block)
NCH = J // T               # 8 chunks per row
NSUB = 4                   # sub-gathers per row
SUBL = L // NSUB           # 1024 frames per sub-gather
GSUB = SUBL // T           # 8 gather groups of 128 frames
EPS = 1e-4

USE_CAST_STORE = True      # gpsimd casting DMA for out stores (else ACT/DVE convert)


def _body(tc, ctx):
    nc = tc.nc
    emb = nc.dram_tensor("unit_embeddings", [BL, J, D], F32, kind="ExternalInput").ap()
    conf = nc.dram_tensor("unit_confidence", [BL, J], F32, kind="ExternalInput").ap()
    mask = nc.dram_tensor("unit_mask", [BL, J], U8, kind="ExternalInput").ap()
    bdry = nc.dram_tensor("boundary_mask", [BL, L], U8, kind="ExternalInput").ap()
    out = nc.dram_tensor("frames", [BL, L, D], F32, kind="ExternalOutput").ap()
    # offset-0 DRAM tensors (indirect DMA requires src offset == 0)
    smoothed = [
        nc.dram_tensor(f"smoothed{r}", [J, D], BF16, kind="Internal").ap()
        for r in range(BL)
    ]

    const_p = ctx.enter_context(tc.tile_pool(name="const", bufs=1))
    coef_p = ctx.enter_context(tc.tile_pool(name="coef", bufs=1))
    e_p = ctx.enter_context(tc.tile_pool(name="e", bufs=6))
    x_p = ctx.enter_context(tc.tile_pool(name="x", bufs=4))
    lt_p = ctx.enter_context(tc.tile_pool(name="lt", bufs=4))
    smn_p = ctx.enter_context(tc.tile_pool(name="smn", bufs=3))
    idx_p = ctx.enter_context(tc.tile_pool(name="idx", bufs=1))
    gout_p = ctx.enter_context(tc.tile_pool(name="gout", bufs=4))
    stg_p = ctx.enter_context(tc.tile_pool(name="stg", bufs=2))
    psum_p = ctx.enter_context(tc.tile_pool(name="psum", bufs=2, space="PSUM"))
    psc_p = ctx.enter_context(tc.tile_pool(name="psc", bufs=3, space="PSUM"))

    ps_ctr = [0]

    def ps_tile(shape, dtype=F32):
        ps_ctr[0] += 1
        return psum_p.tile(shape, dtype, tag="ps", name=f"ps{ps_ctr[0]}")

    # --- constants ---
    ident = const_p.tile([128, 128], F32)
    make_identity(nc, ident[:])
    zeros128 = const_p.tile([128, 128], F32)
    nc.gpsimd.memset(zeros128[:], 0.0)
    # tri128[k, p] = 1 iff k <= p (inclusive-cumsum lhsT)
    tri128 = const_p.tile([128, 128], F32)
    nc.vector.tensor_tensor_scan(
        out=tri128[:], data0=zeros128[:], data1=ident[:],
        initial=0.0, op0=OP.add, op1=OP.add,
    )
    # additive mask: 0 where k <= p (keep), -1e9 where k > p
    trimask = const_p.tile([128, 128], F32)
    nc.vector.tensor_scalar(
        out=trimask[:], in0=tri128[:], scalar1=1e9, scalar2=-1e9,
        op0=OP.mult, op1=OP.add,
    )
    ones_row = const_p.tile([1, 128], F32)
    nc.gpsimd.memset(ones_row[:], 1.0)
    ones_col16 = const_p.tile([16, 1], F32)
    nc.gpsimd.memset(ones_col16[:], 1.0)
    zeros_row256 = const_p.tile([1, 256], F32)
    nc.gpsimd.memset(zeros_row256[:], 0.0)
    # tri16[k, p] = 1 iff k <= p
    tri16 = const_p.tile([16, 16], F32)
    nc.vector.tensor_tensor_scan(
        out=tri16[:], data0=zeros128[:16, :16], data1=ident[:16, :16],
        initial=0.0, op0=OP.add, op1=OP.add,
    )

    # --- coefficients: c (scale), log(1-c) wraps ---
    NC2 = BL * NCH  # 16 columns: r*NCH + c
    c_wrap = coef_p.tile([128, NC2], F32)
    la_wrap = coef_p.tile([128, NC2], F32)
    c_rows = []
    la_rows = []
    for r in range(BL):
        cf = coef_p.tile([1, J], F32, tag=f"cf{r}")
        nc.sync.dma_start(cf[:], conf[r : r + 1, :])
        mk = coef_p.tile([1, J], F32, tag=f"mk{r}")
        nc.gpsimd.dma_start(mk[:], mask[r : r + 1, :])  # u8 -> f32 cast in DMA
        c_r = coef_p.tile([1, J], F32, tag=f"c{r}")
        nc.vector.tensor_scalar(
            out=c_r[:], in0=cf[:], scalar1=EPS, scalar2=1.0 - EPS,
            op0=OP.max, op1=OP.min,
        )
        nc.vector.tensor_tensor(out=c_r[:], in0=c_r[:], in1=mk[:], op=OP.mult)
        a_r = coef_p.tile([1, J], F32, tag=f"a{r}")
        nc.vector.tensor_scalar(
            out=a_r[:], in0=c_r[:], scalar1=-1.0, scalar2=1.0,
            op0=OP.mult, op1=OP.add,
        )
        la_r = coef_p.tile([1, J], F32, tag=f"la{r}")
        nc.scalar.activation(la_r[:], a_r[:], AF.Ln)
        c_rows.append(c_r)
        la_rows.append(la_r)

    # wrap rows into (128, NC2) column layout [p, col] = row[c*128 + p] by
    # staging chunk-per-partition [NC2, 128] via DMA, then PE transpose.
    cstage = coef_p.tile([NC2, T], F32)
    lstage = coef_p.tile([NC2, T], F32)
    for r in range(BL):
        nc.sync.dma_start(cstage[r * NCH : (r + 1) * NCH, :], c_rows[r][:])
        nc.sync.dma_start(lstage[r * NCH : (r + 1) * NCH, :], la_rows[r][:])
    ps_cw = ps_tile([128, NC2])
    nc.tensor.matmul(
        out=ps_cw[:], lhsT=cstage[:], rhs=ident[:NC2, :NC2], start=True, stop=True,
        is_transpose=True,
    )
    nc.vector.tensor_copy(c_wrap[:], ps_cw[:])
    ps_lw = ps_tile([128, NC2])
    nc.tensor.matmul(
        out=ps_lw[:], lhsT=lstage[:], rhs=ident[:NC2, :NC2], start=True, stop=True,
        is_transpose=True,
    )
    nc.scalar.copy(la_wrap[:], ps_lw[:])

    # cs = per-chunk inclusive cumsum of log(a): tri128 @ la_wrap
    ps_cs = ps_tile([128, NC2])
    nc.tensor.matmul(out=ps_cs[:], lhsT=tri128[:], rhs=la_wrap[:], start=True, stop=True)
    negcs_col = coef_p.tile([128, NC2], F32)
    nc.vector.tensor_scalar_mul(negcs_col[:], ps_cs[:], -1.0)
    cs_col = coef_p.tile([128, NC2], F32)
    nc.scalar.copy(cs_col[:], ps_cs[:])
    # cs flattened to partition 0 in (p c) order: [1, p*NC2 + c] = cs_col[p, c]
    # (plain DMA pairs dst's linear walk with src's partition-major walk)
    cs_row0 = coef_p.tile([1, NC2 * 128], F32)
    nc.sync.dma_start(cs_row0[:], cs_col[:])
    cs_row0_v = cs_row0[:].rearrange("one (p c) -> one p c", p=128)
    # decay rows d[i] = exp(cs[i]) for the carry injection, same (p c) order
    d_all = coef_p.tile([1, NC2 * 128], BF16)
    nc.scalar.activation(d_all[:], cs_row0[:], AF.Exp)
    d_all_v = d_all[:].rearrange("one (p c) -> one p c", p=128)

    # --- indices (16-partition wrap for dma_gather): idx16[q, t] = idx[t*16+q]
    idx_rep = []
    for r in range(BL):
        # W[p, q] = bd[q*16 + p] for p in [0,16), q in [0,256)
        w_sb = idx_p.tile([16, 256], F32, tag=f"w{r}")
        for h in range(2):
            vh = idx_p.tile([128, 16], F32, tag=f"vh{r}")
            src_bd = bdry[r, h * 2048 : (h + 1) * 2048].rearrange(
                "(p v) -> p v", p=128
            )
            nc.gpsimd.dma_start(vh[:], src_bd)  # u8 -> f32 cast
            pw = ps_tile([16, 128])
            nc.tensor.matmul(out=pw[:], lhsT=vh[:], rhs=ident[:], start=True, stop=True)
            nc.vector.tensor_copy(w_sb[:, h * 128 : (h + 1) * 128], pw[:])
        pcs = ps_tile([1, 256])
        nc.tensor.matmul(
            out=pcs[:], lhsT=ones_col16[:], rhs=w_sb[:], start=True, stop=True
        )
        cs_sb = idx_p.tile([1, 256], F32, tag=f"cssb{r}")
        nc.vector.tensor_copy(cs_sb[:], pcs[:])
        incl = idx_p.tile([1, 256], F32, tag=f"incl{r}")
        nc.vector.tensor_tensor_scan(
            out=incl[:], data0=cs_sb[:], data1=zeros_row256[:],
            initial=0.0, op0=OP.add, op1=OP.add,
        )
        excl = idx_p.tile([1, 256], F32, tag=f"excl{r}")
        nc.vector.tensor_tensor(out=excl[:], in0=incl[:], in1=cs_sb[:], op=OP.subtract)
        pidx = ps_tile([16, 256])
        nc.tensor.matmul(out=pidx[:], lhsT=tri16[:], rhs=w_sb[:], start=True, stop=False)
        nc.tensor.matmul(
            out=pidx[:], lhsT=ones_row[:, :16], rhs=excl[:], start=False, stop=True
        )
        idxf = idx_p.tile([16, 256], F32, tag=f"idxf{r}")
        nc.vector.tensor_scalar(
            out=idxf[:], in0=pidx[:], scalar1=-1.0, scalar2=0.0, op0=OP.add, op1=OP.max
        )
        nc.vector.tensor_scalar_min(idxf[:], idxf[:], float(J - 1))
        idx16 = idx_p.tile([16, 256], I16, tag=f"idx16{r}")
        nc.vector.tensor_copy(idx16[:], idxf[:])
        rep = idx_p.tile([128, 256], I16, tag=f"rep{r}")
        for k in range(8):
            nc.sync.dma_start(rep[k * 16 : (k + 1) * 16, :], idx16[:])
        idx_rep.append(rep)

    # carry-injection lhsTs: nonzero only in partition 127 (row = decay d_c).
    # Engines can't address partition 127 alone; DMAs can — fill via DMA.
    carry_all = []
    for r in range(BL):
        ca = const_p.tile([128, NCH * 128], BF16, tag=f"clhs{r}", name=f"clhs{r}")
        nc.gpsimd.memset(ca[:], 0.0)
        for c in range(1, NCH):
            nc.sync.dma_start(
                ca[127:128, c * 128 : (c + 1) * 128],
                d_all_v[:, :, r * NCH + c],
            )
        carry_all.append(ca)

    def ema_row(r):
        smn_prev = None
        for c in range(NCH):
            col = r * NCH + c
            e_c = e_p.tile([T, D], F32, tag="e", name=f"e{r}_{c}")
            nc.sync.dma_start(e_c[:], emb[r, c * T : (c + 1) * T, :])
            # x = c * emb (bf16)
            x_c = x_p.tile([T, D], BF16, tag="x", name=f"x{r}_{c}")
            nc.vector.tensor_tensor(
                out=x_c[:], in0=e_c[:],
                in1=c_wrap[:, col : col + 1].to_broadcast([T, D]), op=OP.mult,
            )
            # L^T[j, i] = exp(cs[i] - cs[j]) masked to j <= i.
            # Broadcast this chunk's cs row (strided view) to all partitions.
            ps_rbc = ps_tile([128, 128])
            nc.tensor.matmul(
                out=ps_rbc[:], lhsT=ones_row[:], rhs=cs_row0_v[:, :, col],
                start=True, stop=True,
            )
            rowm = lt_p.tile([128, 128], F32, tag="rowm", name=f"rowm{r}_{c}")
            nc.vector.tensor_tensor(
                out=rowm[:], in0=ps_rbc[:], in1=trimask[:], op=OP.add,
            )
            lt = lt_p.tile([128, 128], BF16, tag="lt", name=f"lt{r}_{c}")
            nc.scalar.activation(
                lt[:], rowm[:], AF.Exp, bias=negcs_col[:, col : col + 1]
            )
            ps_c = psc_p.tile([T, D], F32, tag="psc", name=f"psc{r}_{c}")
            if c == 0:
                nc.tensor.matmul(
                    out=ps_c[:], lhsT=lt[:], rhs=x_c[:], start=True, stop=True
                )
            else:
                nc.tensor.matmul(
                    out=ps_c[:], lhsT=lt[:], rhs=x_c[:], start=True, stop=False
                )
                # carry: decay row (partition 127 of carry_all) times prev chunk
                nc.tensor.matmul(
                    out=ps_c[:], lhsT=carry_all[r][:, c * 128 : (c + 1) * 128],
                    rhs=smn_prev[:], start=False, stop=True,
                )
            smn = smn_p.tile([T, D], BF16, tag="smn", name=f"smn{r}_{c}")
            if c % 2 == 0:
                nc.vector.tensor_copy(smn[:], ps_c[:])
            else:
                nc.scalar.copy(smn[:], ps_c[:])
            nc.sync.dma_start(smoothed[r][c * T : (c + 1) * T, :], smn[:])
            smn_prev = smn

    def gather_sub(r, s):
        gt = gout_p.tile([128, GSUB, D], BF16, tag="gout", name=f"gout{r}_{s}")
        nc.gpsimd.dma_gather(
            out_ap=gt[:],
            in_ap=smoothed[r][:],
            idxs_ap=idx_rep[r][:, s * (SUBL // 16) : (s + 1) * (SUBL // 16)],
            num_idxs=SUBL,
            num_idxs_reg=SUBL,
            elem_size=D,
            queue_num=(r * NSUB + s) % 3 + 1,  # queue 0 blocks; use 1-3
        )
        return gt

    def store_sub(r, s, gt):
        dst = out[r, s * SUBL : (s + 1) * SUBL, :].rearrange(
            "(g p) d -> p g d", p=128
        )
        # A/B on hardware this run: even subs cast in DMA, odd subs convert.
        if USE_CAST_STORE and s % 2 == 0:
            nc.gpsimd.dma_start(dst, gt[:])  # bf16 -> f32 cast in DMA
        else:
            stg = stg_p.tile([128, GSUB, D], F32, tag="stg", name=f"stg{r}_{s}")
            if s % 2 == 0:
                nc.vector.tensor_copy(stg[:], gt[:])
            else:
                nc.scalar.copy(stg[:], gt[:])
            nc.sync.dma_start(dst, stg[:])

    ema_row(0)
    gts0 = [gather_sub(0, s) for s in range(NSUB)]
    ema_row(1)
    gts1 = []
    for s in range(NSUB):
        store_sub(0, s, gts0[s])
        gts1.append(gather_sub(1, s))
    for s in range(NSUB):
        store_sub(1, s, gts1[s])


def _patch_swdge_lane_by_queue():
    """Tile assigns DMASW completion-sem lanes round-robin, queue-blind; the
    HW/sim lock each lane's sem to one SWDGE queue. Pin lane = queue_num so
    multi-queue pool DMAs get consistent lanes."""
    from concourse import bass_isa
    from concourse import tile_sem_assignment as tsa

    if getattr(tsa.TileClockTick, "_ema_queue_patch", False):
        return
    orig = tsa.TileClockTick._assign_tick

    def patched(self, inst):
        if (
            isinstance(inst, bass_isa.AnyDMAInstruction)
            and inst.engine == mybir.EngineType.Pool
            and not isinstance(inst, bass_isa.UserSyncedRemoteDMADescs)
        ):
            self.next_sw_dma_idx = getattr(inst, "queue_num", 0) or 0
        return orig(self, inst)

    tsa.TileClockTick._assign_tick = patched
    tsa.TileClockTick._ema_queue_patch = True


def build():
    _patch_swdge_lane_by_queue()
    nc = bacc.Bacc(
        "TRN2",
        target_bir_lowering=False,
        debug=False,
        enable_asserts=False,
        num_devices=N_CORES,
        num_swdge_queues=4,
        dynamic_dma_scratch_size=16384,
    )
    with tile.TileContext(nc) as tc, ExitStack() as ctx:
        _body(tc, ctx)
    nc.compile()
    return nc


def make_in_maps(inputs):
    emb = np.asarray(inputs["unit_embeddings"], dtype=np.float32)
    conf = np.asarray(inputs["unit_confidence"], dtype=np.float32)
    msk = np.asarray(inputs["unit_mask"]).astype(np.uint8)
    bd = np.asarray(inputs["boundary_mask"]).astype(np.uint8)
    in_maps = []
    for c in range(N_CORES):
        sl = slice(c * BL, (c + 1) * BL)
        in_maps.append(
            {
                "unit_embeddings": np.ascontiguousarray(emb[sl]),
                "unit_confidence": np.ascontiguousarray(conf[sl]),
                "unit_mask": np.ascontiguousarray(msk[sl]),
                "boundary_mask": np.ascontiguousarray(bd[sl]),
            }
        )
    return in_maps


_cached_nc = None


def run(inputs, trace=False):
    global _cached_nc
    if _cached_nc is None:
        _cached_nc = build()
    res = run_bass_kernel_spmd(
        _cached_nc, make_in_maps(inputs), core_ids=list(range(N_CORES)), trace=trace
    )
    full = np.concatenate(
        [res.results[c]["frames"] for c in range(N_CORES)], axis=0
    )
    return full, res


def kernel(**inputs) -> np.ndarray:
    import os

    # Trace capture needs hooks absent outside our dev harness; make sure a
    # stray BASS_TRACE env can't route the grading run down that path.
    prev = os.environ.get("BASS_NEVER_TRACE")
    os.environ["BASS_NEVER_TRACE"] = "1"
    try:
        full, _ = run(inputs, trace=False)
    finally:
        if prev is None:
            os.environ.pop("BASS_NEVER_TRACE", None)
        else:
            os.environ["BASS_NEVER_TRACE"] = prev
    return full
